# revision 5
# baseline (speedup 1.0000x reference)
"""Trainium2 Bass kernel for an AttnBlock++ (GroupNorm -> QKV 1x1 conv ->
full softmax attention over HW tokens -> output projection -> residual/sqrt(2)).

Sharding: data-parallel over batch B=8 across the 8 NeuronCores; attention is
fully independent per batch element, so each core processes one [C, H*W]
feature map with no collectives.

Per-core algorithm (C=256, N=H*W=4096, 32 groups):
  - GroupNorm is folded into the QKV weights: hn = fs*x + fb (per-channel
    affine from group stats), so q/k/v = (W*diag(fs)) @ x + const. The K-side
    constant drops out of softmax exactly (shift invariance); the V-side
    constant is folded into the output-projection bias (attention rows sum
    to 1); only the Q-side constant is applied.
  - Attention runs in fp8 with DoubleRow matmuls (contraction 256 per
    instruction): q/k are fp8e4 in a [128, 2, N] layout, scores are one
    DoubleRow matmul per 128-key tile, exp outputs go straight to fp8e5
    (max score*scale ~ 9.6 -> e^9.6 ~ 15k < 57344 = e5m2 max), att@V and
    the softmax denominator are DoubleRow matmuls over 256-key pairs.
  - Scores are computed transposed (S_T[m, n], keys on partitions); the
    denominator uses a 128-wide all-ones stationary so every partition
    holds the row sum (no gpsimd broadcast needed); division by it commutes
    with the output projection and is applied at the very end.
  - The group rstd is computed with a Newton-iteration rsqrt on the vector
    engine (no Sqrt on the scalar engine), so the scalar engine loads the
    exp activation table exactly once, during the input-DMA window.
  - Emission interleaves score production with att@V / denominator
    consumption at a fixed cadence so the exp spine never starves and the
    PE never head-of-line blocks on the psum score ring.
  - 1/sqrt(2) of the residual is folded into the output weights and biases.
"""

import math

import numpy as np

import concourse.bacc as bacc
import concourse.tile as tile
from concourse.tile import add_dep_helper
from concourse import mybir
from concourse import bass_utils

B, C, H, W = 8, 256, 64, 64
N = H * W  # 4096
G = 32  # groups
GD = C // G  # 8 channels per group
EPS = 1e-6
NCORES = 8
NCH = 2  # channel chunks of 128
NBLK = 8  # query blocks
BLK = 512  # queries per block
MT = 32  # key tiles of 128
NKP = MT // 2  # key pairs of 256 (DoubleRow contraction)
SCALE = float(C) ** -0.5  # 1/16
INV_SQRT2 = float(1.0 / math.sqrt(2.0))
NWARM = 40  # HAM warm-up matmuls issued while the input DMA runs
RSQRT_MAGIC = 0x5F3759DF

FP32 = mybir.dt.float32
BF16 = mybir.dt.bfloat16
U32 = mybir.dt.uint32
F8E4 = mybir.dt.float8e4
F8E5 = mybir.dt.float8e5
AF = mybir.ActivationFunctionType
ALU = mybir.AluOpType
DR = mybir.MatmulPerfMode.DoubleRow


def build_program():
    nc = bacc.Bacc("TRN2", target_bir_lowering=False, debug=False)

    x = nc.dram_tensor("x", [C, N], FP32, kind="ExternalInput").ap()
    wqT = nc.dram_tensor("wqT", [C, C], FP32, kind="ExternalInput").ap()
    wkT = nc.dram_tensor("wkT", [C, C], FP32, kind="ExternalInput").ap()
    wvT = nc.dram_tensor("wvT", [C, C], FP32, kind="ExternalInput").ap()
    woT = nc.dram_tensor("woT", [C, C], FP32, kind="ExternalInput").ap()
    # wvoT[c_in, c_out] = (w_v.T @ w_o.T)/sqrt(2): output projection folded
    # into the V weights on the host (fs scaling still applied on device)
    wvoT = nc.dram_tensor("wvoT", [C, C], FP32, kind="ExternalInput").ap()
    bq = nc.dram_tensor("bq", [C, 1], FP32, kind="ExternalInput").ap()
    bv = nc.dram_tensor("bv", [C, 1], FP32, kind="ExternalInput").ap()
    bo = nc.dram_tensor("bo", [C, 1], FP32, kind="ExternalInput").ap()
    gns = nc.dram_tensor("gns", [C, 1], FP32, kind="ExternalInput").ap()
    gnb = nc.dram_tensor("gnb", [C, 1], FP32, kind="ExternalInput").ap()
    # ind16[c, g] = 1/8 if c//8 == g else 0 (group-average over channels)
    ind16 = nc.dram_tensor("ind16", [128, 16], FP32, kind="ExternalInput").ap()
    # bcast16[g, c] = 1 if c//8 == g else 0 (broadcast group value to channels)
    bcast16 = nc.dram_tensor("bcast16", [16, 128], FP32, kind="ExternalInput").ap()
    y = nc.dram_tensor("y", [C, N], FP32, kind="ExternalOutput").ap()

    with tile.TileContext(nc) as tc:
        with (
            tc.tile_pool(name="persist", bufs=1) as P,
            tc.tile_pool(name="work", bufs=2) as WK,
        ):
            # ---------------- constants + exp-table preload ----------------
            junk_bf = P.tile([128, 128], BF16, tag="junk")
            nc.gpsimd.memset(junk_bf, 0.0)
            ones_dr = P.tile([128, 2, 128], F8E5, tag="ones_dr")
            nc.vector.memset(ones_dr, 1.0)
            eps16 = P.tile([16, 1], FP32, tag="eps16")
            nc.vector.memset(eps16, EPS)
            # dummy Exp: loads the scalar-engine exp table set during the
            # DMA window; Identity/Copy/Square used later live in the same
            # set, so no further table loads ever happen.
            tbl_warm = P.tile([16, 1], FP32, tag="tblwarm")
            nc.scalar.activation(out=tbl_warm, in_=eps16, func=AF.Exp)

            # ---------------- input DMA (all dispatched from gpsimd) --------
            # Order: tiny tensors + score-side weights first (they gate the
            # affine -> k/q projections right after stats), then x (stats
            # critical path), then the value/output-side weights.
            def load2(ap, name, width):
                ts = []
                for j in range(NCH):
                    t = P.tile([128, width], FP32, tag=f"{name}{j}", name=f"{name}{j}")
                    nc.gpsimd.dma_start(out=t, in_=ap[j * 128:(j + 1) * 128, :])
                    ts.append(t)
                return ts

            ind16_sb = P.tile([128, 16], FP32, tag="ind16")
            nc.gpsimd.dma_start(out=ind16_sb, in_=ind16)
            bcast16_sb = P.tile([16, 128], FP32, tag="bcast16")
            nc.gpsimd.dma_start(out=bcast16_sb, in_=bcast16)
            gns_sb = load2(gns, "gns", 1)
            gnb_sb = load2(gnb, "gnb", 1)
            bq_sb = load2(bq, "bq", 1)
            wkT_sb = load2(wkT, "wkT", C)
            wqT_sb = load2(wqT, "wqT", C)

            x_sb = []
            for j in range(NCH):
                t = P.tile([128, N], FP32, tag=f"x{j}", name=f"x{j}")
                x_sb.append(t)
            xdmas = []
            for p in range(4):  # [128, 1024] pieces: 4KB per partition row
                for j in (1, 0):
                    cs = slice(p * 1024, (p + 1) * 1024)
                    xdmas.append(nc.gpsimd.dma_start(
                        out=x_sb[j][:, cs], in_=x[j * 128:(j + 1) * 128, cs]
                    ))

            wvoT_sb = load2(wvoT, "wvoT", C)
            bv_sb = load2(bv, "bv", 1)
            bo_sb = load2(bo, "bo", 1)
            wvT_sb = load2(wvT, "wvT", C)
            woT_sb = load2(woT, "woT", C)

            with tc.tile_pool(name="psum_s", bufs=1, space="PSUM") as PSS, \
                 tc.tile_pool(name="psum_av", bufs=1, space="PSUM") as PSAV, \
                 tc.tile_pool(name="psum_d", bufs=1, space="PSUM") as PSD:
                # One psum layout for the whole kernel (8 banks):
                #   s2[0..2] (2 banks each: 3-deep score ring) | av | d
                s2 = [
                    PSS.tile([128, 2 * BLK], FP32, tag=f"s2_{r}", name=f"s2_{r}")
                    for r in range(3)
                ]
                av_slot = PSAV.tile([128, BLK], FP32, tag="av", name="av_slot")

                # HAM warm-up: keep the PE active from the start of the DMA
                # window through phase 0, so the array never throttles to
                # half clock before the projection/attention stream begins.
                junk_cnt = [0]

                def junk(dep=None, n=1):
                    for _ in range(n):
                        jm = nc.tensor.matmul(
                            av_slot[:, 0:128], junk_bf, junk_bf,
                            start=True, stop=True,
                        )
                        junk_cnt[0] += 1
                        if dep is not None:
                            add_dep_helper(jm.ins, dep.ins, sync=True,
                                           reason="HAM trickle")

                junk(n=NWARM)
                for dins in xdmas:
                    junk(dep=dins, n=4)

                # ---------------- phase 0: group stats -> folded affine -----
                # chunk 0 stats on the vector engine (bn_stats),
                # chunk 1 stats on the scalar engine (accum of x and x^2);
                # x is cast to the fp8e4 DoubleRow layout on the way.
                x_f8 = P.tile([128, NCH, N], F8E4, tag="xf8")
                stats = WK.tile([128, 8, 6], FP32, tag="bnstats")
                xsum_p = WK.tile([128, 8], FP32, tag="xsump")
                for p in range(4):
                    ps = slice(p * 1024, (p + 1) * 1024)
                    a1 = nc.scalar.activation(
                        out=x_f8[:, 1, ps], in_=x_sb[1][:, ps], func=AF.Copy,
                        accum_out=xsum_p[:, p:p + 1],
                    )
                    sq_scr = WK.tile([128, 1024], BF16, tag="sqscr")
                    a2 = nc.scalar.activation(
                        out=sq_scr, in_=x_sb[1][:, ps], func=AF.Square,
                        accum_out=xsum_p[:, 4 + p:5 + p],
                    )
                    b1 = nc.vector.bn_stats(
                        out=stats[:, 2 * p, :],
                        in_=x_sb[0][:, p * 1024:p * 1024 + 512],
                    )
                    b2 = nc.vector.bn_stats(
                        out=stats[:, 2 * p + 1, :],
                        in_=x_sb[0][:, p * 1024 + 512:(p + 1) * 1024],
                    )
                    c1 = nc.vector.tensor_copy(
                        out=x_f8[:, 0, ps], in_=x_sb[0][:, ps]
                    )
                    junk(dep=a1)
                    junk(dep=a2)
                    junk(dep=b2)
                    junk(dep=c1)

                # chunk 0: [mean, E[x^2]] from bn aggregation
                mv = WK.tile([128, 2], FP32, tag="bnmv")
                nc.vector.bn_aggr(out=mv, in_=stats)
                t2_0 = WK.tile([128, 2], FP32, tag="chstat0")
                nc.vector.tensor_copy(out=t2_0[:, 0:1], in_=mv[:, 0:1])
                sq = WK.tile([128, 1], FP32, tag="chsq")
                nc.vector.tensor_mul(out=sq, in0=mv[:, 0:1], in1=mv[:, 0:1])
                nc.vector.tensor_add(out=t2_0[:, 1:2], in0=mv[:, 1:2], in1=sq)
                # chunk 1: sums -> [mean, E[x^2]]
                t2_1 = WK.tile([128, 2], FP32, tag="chstat1")
                sab = WK.tile([128, 4], FP32, tag="sab")
                nc.vector.tensor_add(
                    out=sab[:, 0:2], in0=xsum_p[:, 0:2], in1=xsum_p[:, 2:4]
                )
                nc.vector.tensor_add(
                    out=sab[:, 2:4], in0=xsum_p[:, 4:6], in1=xsum_p[:, 6:8]
                )
                nc.vector.tensor_add(
                    out=t2_1[:, 0:1], in0=sab[:, 0:1], in1=sab[:, 1:2]
                )
                nc.vector.tensor_add(
                    out=t2_1[:, 1:2], in0=sab[:, 2:3], in1=sab[:, 3:4]
                )
                jm0 = nc.vector.tensor_scalar_mul(out=t2_1, in0=t2_1, scalar1=1.0 / N)
                junk(dep=jm0, n=2)

                # group stats: [16, 2] per chunk in adjacent psum columns
                t2 = [t2_0, t2_1]
                for j in range(NCH):
                    nc.tensor.matmul(
                        s2[0][0:16, 2 * j:2 * j + 2], ind16_sb, t2[j],
                        start=True, stop=True,
                    )
                g24 = WK.tile([16, 2, 2], FP32, tag="g24")
                nc.vector.tensor_copy(
                    out=g24.rearrange("p a b -> p (a b)"), in_=s2[0][0:16, 0:4]
                )
                mean2 = g24[:, :, 0]  # [16, 2]: group mean per chunk
                ex2 = g24[:, :, 1]  # [16, 2]: group E[x^2] per chunk

                # rstd = rsqrt(var + eps) via bit-trick seed + 2 Newton steps
                # (keeps Sqrt off the scalar engine: exp table stays loaded)
                msq = WK.tile([16, 2], FP32, tag="msq")
                nc.vector.tensor_mul(out=msq, in0=mean2, in1=mean2)
                gvp = WK.tile([16, 2], FP32, tag="gvp")
                nc.vector.tensor_sub(out=gvp, in0=ex2, in1=msq)
                jm1 = nc.vector.tensor_scalar_add(out=gvp, in0=gvp, scalar1=EPS)
                hv = WK.tile([16, 2], FP32, tag="hv")
                nc.vector.tensor_scalar_mul(out=hv, in0=gvp, scalar1=0.5)
                yr = WK.tile([16, 2], FP32, tag="yrsqrt")
                yu = yr.bitcast(U32)
                # magic - (v>>1) computed as (~(v>>1)) - (0xFFFFFFFF - magic):
                # both steps stay in range (DVE integer ops saturate, they
                # don't wrap)
                nc.vector.tensor_scalar(
                    out=yu, in0=gvp.bitcast(U32), scalar1=1,
                    scalar2=0xFFFFFFFF,
                    op0=ALU.logical_shift_right, op1=ALU.bitwise_xor,
                )
                nc.vector.tensor_scalar_sub(
                    out=yu, in0=yu, scalar1=0xFFFFFFFF - RSQRT_MAGIC
                )
                junk(dep=jm1, n=2)
                for it in range(2):
                    aa = WK.tile([16, 2], FP32, tag=f"nra{it}")
                    nc.vector.tensor_mul(out=aa, in0=yr, in1=yr)
                    nc.vector.tensor_mul(out=aa, in0=aa, in1=hv)
                    nc.vector.tensor_scalar(
                        out=aa, in0=aa, scalar1=-1.0, scalar2=1.5,
                        op0=ALU.mult, op1=ALU.add,
                    )
                    jm2 = nc.vector.tensor_mul(out=yr, in0=yr, in1=aa)
                    junk(dep=jm2)

                fs_sb, fb_sb = [], []
                for j in range(NCH):
                    gm_r = WK.tile([16, 2], FP32, tag=f"gmr{j}")
                    nc.vector.tensor_copy(out=gm_r[:, 0:1], in_=mean2[:, j:j + 1])
                    nc.vector.tensor_copy(out=gm_r[:, 1:2], in_=yr[:, j:j + 1])
                    ps_bc = s2[1][:, 2 * j:2 * j + 2]
                    nc.tensor.matmul(ps_bc, bcast16_sb, gm_r, start=True, stop=True)
                    mbrb = WK.tile([128, 2], FP32, tag=f"mbrb{j}")
                    nc.vector.tensor_copy(out=mbrb, in_=ps_bc)
                    fs = P.tile([128, 1], FP32, tag=f"fs{j}", name=f"fs{j}")
                    nc.vector.tensor_mul(out=fs, in0=gns_sb[j], in1=mbrb[:, 1:2])
                    tmp = WK.tile([128, 1], FP32, tag=f"fbt{j}")
                    nc.vector.tensor_mul(out=tmp, in0=mbrb[:, 0:1], in1=fs)
                    fb = P.tile([128, 1], FP32, tag=f"fb{j}", name=f"fb{j}")
                    nc.vector.tensor_sub(out=fb, in0=gnb_sb[j], in1=tmp)
                    fs_sb.append(fs)
                    fb_sb.append(fb)

                # fp8e4 DoubleRow weights: w'[c_in, c_out] = wT*fs[c_in]*16
                # (x16 keeps typical weight magnitudes out of the e4m3
                # subnormal range; the 1/16 is folded into the psum drains).
                # k first: it gates the score stream for all tiles.
                wq_dr = P.tile([128, NCH, C], F8E4, tag="wqdr")
                wk_dr = P.tile([128, NCH, C], F8E4, tag="wkdr")
                wvo_dr = P.tile([128, NCH, C], F8E4, tag="wvodr")
                for wdst, wsrc in ((wk_dr, wkT_sb), (wq_dr, wqT_sb),
                                   (wvo_dr, wvoT_sb)):
                    for j in range(NCH):
                        jw = nc.vector.tensor_scalar(
                            out=wdst[:, j, :], in0=wsrc[j],
                            scalar1=fs_sb[j], scalar2=16.0,
                            op0=ALU.mult, op1=ALU.mult,
                        )
                        junk(dep=jw)

                # cQ = Wq @ fb + bq (gates the q drains); cV/bo_eff later.
                cq_sb, cv_sb, boe_sb = [], [], []
                rot_t = [0]

                def tiny_mm(wT, rhs2):
                    ps_c = PSD.tile([128, BLK], FP32, tag="d",
                                    name=f"tiny{rot_t[0]}")[:, 300:301]
                    rot_t[0] += 1
                    nc.tensor.matmul(
                        ps_c, wT[0][:, :], rhs2[0], start=True, stop=False,
                    )
                    nc.tensor.matmul(
                        ps_c, wT[1][:, :], rhs2[1], start=False, stop=True,
                    )
                    return ps_c

                for o in range(NCH):
                    ps_c = tiny_mm(
                        [wqT_sb[0][:, o * 128:(o + 1) * 128],
                         wqT_sb[1][:, o * 128:(o + 1) * 128]],
                        fb_sb,
                    )
                    t = P.tile([128, 1], FP32, tag=f"cq{o}", name=f"cq{o}")
                    nc.vector.tensor_add(out=t, in0=ps_c, in1=bq_sb[o])
                    cq_sb.append(t)

                def emit_cv():
                    for o in range(NCH):
                        ps_c = tiny_mm(
                            [wvT_sb[0][:, o * 128:(o + 1) * 128],
                             wvT_sb[1][:, o * 128:(o + 1) * 128]],
                            fb_sb,
                        )
                        t = P.tile([128, 1], FP32, tag=f"cv{o}", name=f"cv{o}")
                        nc.vector.tensor_add(out=t, in0=ps_c, in1=bv_sb[o])
                        cv_sb.append(t)

                def emit_boe():
                    for o in range(NCH):
                        ps_c = tiny_mm(
                            [woT_sb[0][:, o * 128:(o + 1) * 128],
                             woT_sb[1][:, o * 128:(o + 1) * 128]],
                            cv_sb,
                        )
                        t = P.tile([128, 1], FP32, tag=f"boe{o}", name=f"boe{o}")
                        nc.vector.tensor_add(out=t, in0=ps_c, in1=bo_sb[o])
                        boe_sb.append(t)

                # ---------------- phases 1+2: fused projection/attention ----
                q_f8 = P.tile([128, NCH, N], F8E4, tag="qf8")
                k_f8 = P.tile([128, NCH, N], F8E4, tag="kf8")
                vt_f8 = P.tile([128, MT, C], F8E4, tag="vt")
                e_buf = [
                    P.tile([128, MT, BLK], F8E5, tag=f"ebuf{p}", name=f"ebuf{p}")
                    for p in range(2)
                ]
                e_flat = [t.rearrange("p a b -> p (a b)") for t in e_buf]
                vt_flat = vt_f8.rearrange("p a b -> p (a b)")

                rslot = [0]
                cur_reg = [None]

                def next_region():
                    r = s2[rslot[0] % 3]
                    rslot[0] += 1
                    return r

                def proj_pair(which, nb):
                    # both chunks of one token block claim one full region;
                    # the two drains run on two engines in parallel (the
                    # scalar engine only before the exp spine starts).
                    cs = slice(nb * BLK, (nb + 1) * BLK)
                    reg = next_region()
                    wdr = wq_dr if which == "q" else wk_dr
                    for o in range(NCH):
                        ps = reg[:, o * BLK:(o + 1) * BLK]
                        nc.tensor.matmul(
                            ps, wdr[:, :, o * 128:(o + 1) * 128], x_f8[:, :, cs],
                            start=True, stop=True, perf_mode=DR,
                        )
                        if which == "q":
                            if nb == 0 and o == 0:
                                nc.scalar.activation(
                                    out=q_f8[:, o, cs], in_=ps, func=AF.Identity,
                                    bias=cq_sb[o], scale=1.0 / 16.0,
                                )
                            else:
                                nc.vector.tensor_scalar(
                                    out=q_f8[:, o, cs], in0=ps,
                                    scalar1=1.0 / 16.0, scalar2=cq_sb[o],
                                    op0=ALU.mult, op1=ALU.add,
                                )
                        else:
                            if nb == 0 and o == 0:
                                nc.scalar.activation(
                                    out=k_f8[:, o, cs], in_=ps, func=AF.Copy,
                                    scale=1.0 / 16.0,
                                )
                            else:
                                nc.vector.tensor_scalar_mul(
                                    out=k_f8[:, o, cs], in0=ps, scalar1=1.0 / 16.0
                                )

                def emit_score(nb, k):
                    # one DoubleRow matmul per 128-key tile (contraction 256);
                    # exp drains two tiles at once from a [128, 1024] region.
                    ms = slice(k * 128, (k + 1) * 128)
                    cs = slice(nb * BLK, (nb + 1) * BLK)
                    half = k % 2
                    if half == 0:
                        cur_reg[0] = next_region()
                    reg = cur_reg[0]
                    nc.tensor.matmul(
                        reg[:, half * BLK:(half + 1) * BLK],
                        k_f8[:, :, ms], q_f8[:, :, cs],
                        start=True, stop=True, perf_mode=DR,
                    )
                    if half == 1:
                        nc.scalar.activation(
                            out=e_flat[nb % 2][:, (k - 1) * BLK:(k + 1) * BLK],
                            in_=reg, func=AF.Exp, scale=SCALE,
                        )

                def v_pair(i):
                    # two v tiles into the av bank, one [128, 512] drain
                    for h in range(2):
                        k = 2 * i + h
                        ms = slice(k * 128, (k + 1) * 128)
                        nc.tensor.matmul(
                            av_slot[:, h * C:(h + 1) * C], x_f8[:, :, ms],
                            wvo_dr, start=True, stop=True, perf_mode=DR,
                        )
                    nc.vector.tensor_scalar_mul(
                        out=vt_flat[:, 2 * i * C:(2 * i + 2) * C],
                        in0=av_slot, scalar1=1.0 / 16.0,
                    )

                # ---- epoch 0: block-0 scores/exp + all projections ----
                proj_pair("k", 0)
                proj_pair("q", 0)
                for k in range(MT):
                    emit_score(0, k)
                    if k % 2 == 1:
                        v_pair(k // 2)
                    if k % 4 == 3 and k < 28:
                        proj_pair("k", (k + 1) // 4)
                    if k == 25:
                        emit_cv()
                    if k == 27:
                        emit_boe()
                    if k == 29:
                        proj_pair("q", 1)
                    if k == 31:
                        proj_pair("q", 2)

                # ---- epochs 1..8: consume block c = j-1, produce block j ----
                av_sb_cur = [None, None]
                rb_cur = [None]
                xb_cur = [None, None]

                def av_group(j, o, slot):
                    eb = e_buf[(j - 1) % 2]
                    for kp in range(NKP):
                        nc.tensor.matmul(
                            slot,
                            vt_f8[:, 2 * kp:2 * kp + 2, o * 128:(o + 1) * 128],
                            eb[:, 2 * kp:2 * kp + 2, :],
                            start=(kp == 0), stop=(kp == NKP - 1),
                            perf_mode=DR,
                        )

                def av_drain(o, slot):
                    t = WK.tile([128, BLK], BF16, tag=f"avsb{o}", name=f"avsb{o}")
                    nc.vector.tensor_copy(out=t, in_=slot)
                    av_sb_cur[o] = t

                def d_mm(j, i, ps_d):
                    eb = e_buf[(j - 1) % 2]
                    nc.tensor.matmul(
                        ps_d, ones_dr, eb[:, 2 * i:2 * i + 2, :],
                        start=(i == 0), stop=(i == NKP - 1), perf_mode=DR,
                    )

                def d_recip(ps_d):
                    rb = WK.tile([128, BLK], FP32, tag="rbsb")
                    nc.vector.reciprocal_approx_fast(rb, ps_d)
                    rb_cur[0] = rb

                def xb_make(c, o):
                    # xb = (x + bo_eff) / sqrt(2)
                    ccs = slice(c * BLK, (c + 1) * BLK)
                    xb_t = WK.tile([128, BLK], FP32, tag=f"xbt{o}")
                    nc.vector.tensor_scalar(
                        out=xb_t, in0=x_sb[o][:, ccs],
                        scalar1=boe_sb[o], scalar2=INV_SQRT2,
                        op0=ALU.add, op1=ALU.mult,
                    )
                    xb_cur[o] = xb_t

                def y_emit(c, o):
                    # y = x/sqrt2 + bo_eff/sqrt2 + AV'/denom
                    ccs = slice(c * BLK, (c + 1) * BLK)
                    t_t = WK.tile([128, BLK], FP32, tag=f"tt{o}")
                    nc.vector.tensor_tensor(
                        out=t_t, in0=av_sb_cur[o], in1=rb_cur[0], op=ALU.mult
                    )
                    y_t = WK.tile([128, BLK], FP32, tag=f"yt{o}")
                    nc.vector.tensor_add(out=y_t, in0=t_t, in1=xb_cur[o])
                    nc.gpsimd.dma_start(out=y[o * 128:(o + 1) * 128, ccs], in_=y_t)

                for j in range(1, NBLK + 1):
                    c = j - 1  # consumer block
                    last = j == NBLK
                    ps_d = PSD.tile([128, BLK], FP32, tag="d", name=f"d{j}")
                    if last:
                        # tail: denominator first (progressively follows the
                        # last exps), then av chunk 0 -> y0 -> av chunk 1 -> y1
                        for i in range(NKP):
                            d_mm(j, i, ps_d)
                        d_recip(ps_d)
                        av_group(j, 0, av_slot)
                        av_drain(0, av_slot)
                        xb_make(c, 0)
                        y_emit(c, 0)
                        ps_av1 = PSD.tile([128, BLK], FP32, tag="d", name="avlast1")
                        av_group(j, 1, ps_av1)
                        av_drain(1, ps_av1)
                        xb_make(c, 1)
                        y_emit(c, 1)
                        break

                    # interleaved cadence: per group g, 4 score matmuls
                    # (= 2 exps) against 4 av matmuls + 2 denominator
                    # matmuls, so producers and consumers zip by design.
                    eb = e_buf[(j - 1) % 2]
                    for g in range(8):
                        for t in range(4):
                            emit_score(j, 4 * g + t)
                        o = 0 if g < 4 else 1
                        kb = 4 * (g % 4)
                        slot = av_slot
                        for kp in range(kb, kb + 4):
                            nc.tensor.matmul(
                                slot,
                                vt_f8[:, 2 * kp:2 * kp + 2, o * 128:(o + 1) * 128],
                                eb[:, 2 * kp:2 * kp + 2, :],
                                start=(kp == 0), stop=(kp == NKP - 1),
                                perf_mode=DR,
                            )
                        for i in (2 * g, 2 * g + 1):
                            d_mm(j, i, ps_d)
                        if g == 0 and j <= 5:
                            proj_pair("q", j + 2)
                        if g == 3:
                            av_drain(0, av_slot)
                            xb_make(c, 0)
                        if g == 7:
                            av_drain(1, av_slot)
                            xb_make(c, 1)
                    d_recip(ps_d)
                    y_emit(c, 0)
                    y_emit(c, 1)

    nc.compile()
    return nc


_PROGRAM = None


def _get_program():
    global _PROGRAM
    if _PROGRAM is None:
        _PROGRAM = build_program()
    return _PROGRAM


def make_in_maps(inputs):
    x = np.ascontiguousarray(np.asarray(inputs["x"], dtype=np.float32))
    shared = {
        "wqT": np.ascontiguousarray(np.asarray(inputs["w_q"], np.float32).T),
        "wkT": np.ascontiguousarray(np.asarray(inputs["w_k"], np.float32).T),
        "wvT": np.ascontiguousarray(np.asarray(inputs["w_v"], np.float32).T),
        "woT": np.ascontiguousarray(np.asarray(inputs["w_o"], np.float32).T),
        "wvoT": np.ascontiguousarray(
            (np.asarray(inputs["w_v"], np.float32).T
             @ np.asarray(inputs["w_o"], np.float32).T) * INV_SQRT2
        ),
        "bq": np.asarray(inputs["b_q"], np.float32).reshape(C, 1).copy(),
        "bv": np.asarray(inputs["b_v"], np.float32).reshape(C, 1).copy(),
        "bo": np.asarray(inputs["b_o"], np.float32).reshape(C, 1).copy(),
        "gns": np.asarray(inputs["gn_scale"], np.float32).reshape(C, 1).copy(),
        "gnb": np.asarray(inputs["gn_bias"], np.float32).reshape(C, 1).copy(),
        "ind16": (
            (np.arange(128)[:, None] // GD == np.arange(16)[None, :]) / GD
        ).astype(np.float32),
        "bcast16": (
            np.arange(16)[:, None] == np.arange(128)[None, :] // GD
        ).astype(np.float32),
    }
    in_maps = []
    for i in range(NCORES):
        m = dict(shared)
        m["x"] = np.ascontiguousarray(x[i].reshape(C, N))
        in_maps.append(m)
    return in_maps


def run(inputs, trace=False, trace_cores=None):
    nc = _get_program()
    in_maps = make_in_maps(inputs)
    res = bass_utils.run_bass_kernel_spmd(
        nc, in_maps, core_ids=list(range(NCORES)), trace=trace,
        trace_cores=trace_cores,
    )
    out = np.stack(
        [res.results[i]["y"].reshape(C, H, W) for i in range(NCORES)]
    ).astype(np.float32)
    return out, res


def kernel(**inputs) -> np.ndarray:
    out, _ = run(inputs, trace=False)
    return out


# revision 11
# speedup vs baseline: 1.0216x; 1.0216x over previous
"""Trainium2 Bass kernel for an AttnBlock++ (GroupNorm -> QKV 1x1 conv ->
full softmax attention over HW tokens -> output projection -> residual/sqrt(2)).

Sharding: data-parallel over batch B=8 across the 8 NeuronCores; attention is
fully independent per batch element, so each core processes one [C, H*W]
feature map with no collectives.

Per-core algorithm (C=256, N=H*W=4096, 32 groups):
  - GroupNorm is folded into the QKV weights: hn = fs*x + fb (per-channel
    affine from group stats), so q/k/v = (W*diag(fs)) @ x + const. The K-side
    constant drops out of softmax exactly (shift invariance); the V-side
    constant is folded into the output-projection bias (attention rows sum
    to 1); only the Q-side constant is applied.
  - Attention runs in fp8 with DoubleRow matmuls (contraction 256 per
    instruction): q/k are fp8e4 in a [128, 2, N] layout, scores are one
    DoubleRow matmul per 128-key tile, exp outputs go straight to fp8e5
    (max score*scale ~ 9.6 -> e^9.6 ~ 15k < 57344 = e5m2 max), att@V and
    the softmax denominator are DoubleRow matmuls over 256-key pairs.
  - Scores are computed transposed (S_T[m, n], keys on partitions); the
    denominator uses a 128-wide all-ones stationary so every partition
    holds the row sum (no gpsimd broadcast needed); division by it commutes
    with the output projection and is applied at the very end.
  - The group rstd is computed with a Newton-iteration rsqrt on the vector
    engine (no Sqrt on the scalar engine), so the scalar engine loads the
    exp activation table exactly once, during the input-DMA window.
  - Emission interleaves score production with att@V / denominator
    consumption at a fixed cadence so the exp spine never starves and the
    PE never head-of-line blocks on the psum score ring.
  - 1/sqrt(2) of the residual is folded into the output weights and biases.
"""

import math

import numpy as np

import concourse.bacc as bacc
import concourse.tile as tile
from concourse.tile import add_dep_helper
from concourse import mybir
from concourse import bass_utils

B, C, H, W = 8, 256, 64, 64
N = H * W  # 4096
G = 32  # groups
GD = C // G  # 8 channels per group
EPS = 1e-6
NCORES = 8
NCH = 2  # channel chunks of 128
NBLK = 8  # query blocks
BLK = 512  # queries per block
MT = 32  # key tiles of 128
NKP = MT // 2  # key pairs of 256 (DoubleRow contraction)
SCALE = float(C) ** -0.5  # 1/16
INV_SQRT2 = float(1.0 / math.sqrt(2.0))
NWARM = 40  # HAM warm-up matmuls issued while the input DMA runs
RSQRT_MAGIC = 0x5F3759DF

FP32 = mybir.dt.float32
BF16 = mybir.dt.bfloat16
U32 = mybir.dt.uint32
F8E4 = mybir.dt.float8e4
F8E5 = mybir.dt.float8e5
AF = mybir.ActivationFunctionType
ALU = mybir.AluOpType
DR = mybir.MatmulPerfMode.DoubleRow


def build_program():
    nc = bacc.Bacc("TRN2", target_bir_lowering=False, debug=False)

    x = nc.dram_tensor("x", [C, N], FP32, kind="ExternalInput").ap()
    wqT = nc.dram_tensor("wqT", [C, C], FP32, kind="ExternalInput").ap()
    wkT = nc.dram_tensor("wkT", [C, C], FP32, kind="ExternalInput").ap()
    wvT = nc.dram_tensor("wvT", [C, C], FP32, kind="ExternalInput").ap()
    woT = nc.dram_tensor("woT", [C, C], FP32, kind="ExternalInput").ap()
    # wvoT[c_in, c_out] = (w_v.T @ w_o.T)/sqrt(2): output projection folded
    # into the V weights on the host (fs scaling still applied on device)
    wvoT = nc.dram_tensor("wvoT", [C, C], FP32, kind="ExternalInput").ap()
    bq = nc.dram_tensor("bq", [C, 1], FP32, kind="ExternalInput").ap()
    bv = nc.dram_tensor("bv", [C, 1], FP32, kind="ExternalInput").ap()
    bo = nc.dram_tensor("bo", [C, 1], FP32, kind="ExternalInput").ap()
    gns = nc.dram_tensor("gns", [C, 1], FP32, kind="ExternalInput").ap()
    gnb = nc.dram_tensor("gnb", [C, 1], FP32, kind="ExternalInput").ap()
    # ind16[c, g] = 1/8 if c//8 == g else 0 (group-average over channels)
    ind16 = nc.dram_tensor("ind16", [128, 16], FP32, kind="ExternalInput").ap()
    # bcast16[g, c] = 1 if c//8 == g else 0 (broadcast group value to channels)
    bcast16 = nc.dram_tensor("bcast16", [16, 128], FP32, kind="ExternalInput").ap()
    y = nc.dram_tensor("y", [C, N], FP32, kind="ExternalOutput").ap()

    with tile.TileContext(nc) as tc:
        with (
            tc.tile_pool(name="persist", bufs=1) as P,
            tc.tile_pool(name="work", bufs=2) as WK,
        ):
            # ---------------- constants + exp-table preload ----------------
            junk_bf = P.tile([128, 128], BF16, tag="junk")
            nc.gpsimd.memset(junk_bf, 0.0)
            junk_mv = P.tile([128, 512], BF16, tag="junkmv")
            nc.gpsimd.memset(junk_mv, 0.0)
            ones_dr = P.tile([128, 2, 128], F8E5, tag="ones_dr")
            nc.vector.memset(ones_dr, 1.0)
            eps16 = P.tile([16, 1], FP32, tag="eps16")
            nc.vector.memset(eps16, EPS)
            # dummy Exp: loads the scalar-engine exp table set during the
            # DMA window; Identity/Copy/Square used later live in the same
            # set, so no further table loads ever happen.
            tbl_warm = P.tile([16, 1], FP32, tag="tblwarm")
            nc.scalar.activation(out=tbl_warm, in_=eps16, func=AF.Exp)

            # ---------------- input DMA ----------------
            # x is dispatched first, from the sync engine (the stats critical
            # path); everything else goes through the gpsimd sequencer in
            # parallel, ordered by when each tensor is needed.
            x_sb = []
            for j in range(NCH):
                t = P.tile([128, N], FP32, tag=f"x{j}", name=f"x{j}")
                x_sb.append(t)
            xdmas = []
            for p in range(4):  # [128, 1024] pieces: 4KB per partition row
                for j in (1, 0):
                    cs = slice(p * 1024, (p + 1) * 1024)
                    xdmas.append(nc.sync.dma_start(
                        out=x_sb[j][:, cs], in_=x[j * 128:(j + 1) * 128, cs]
                    ))

            def load2(ap, name, width):
                ts = []
                for j in range(NCH):
                    t = P.tile([128, width], FP32, tag=f"{name}{j}", name=f"{name}{j}")
                    nc.gpsimd.dma_start(out=t, in_=ap[j * 128:(j + 1) * 128, :])
                    ts.append(t)
                return ts

            ind16_sb = P.tile([128, 16], FP32, tag="ind16")
            nc.gpsimd.dma_start(out=ind16_sb, in_=ind16)
            bcast16_sb = P.tile([16, 128], FP32, tag="bcast16")
            nc.gpsimd.dma_start(out=bcast16_sb, in_=bcast16)
            gns_sb = load2(gns, "gns", 1)
            gnb_sb = load2(gnb, "gnb", 1)
            bq_sb = load2(bq, "bq", 1)
            bv_sb = load2(bv, "bv", 1)
            bo_sb = load2(bo, "bo", 1)
            wqT_sb = load2(wqT, "wqT", C)
            wkT_sb = load2(wkT, "wkT", C)
            wvoT_sb = load2(wvoT, "wvoT", C)
            wvT_sb = load2(wvT, "wvT", C)
            woT_sb = load2(woT, "woT", C)

            with tc.tile_pool(name="psum_s", bufs=1, space="PSUM") as PSS, \
                 tc.tile_pool(name="psum_av", bufs=1, space="PSUM") as PSAV, \
                 tc.tile_pool(name="psum_d", bufs=1, space="PSUM") as PSD:
                # One psum layout for the whole kernel (8 banks):
                #   s2[0..2] (2 banks each: 3-deep score ring) | av | d
                s2 = [
                    PSS.tile([128, 2 * BLK], FP32, tag=f"s2_{r}", name=f"s2_{r}")
                    for r in range(3)
                ]
                av_slot = PSAV.tile([128, BLK], FP32, tag="av", name="av_slot")

                # HAM warm-up: keep the PE active from the start of the DMA
                # window through phase 0, so the array never throttles to
                # half clock before the projection/attention stream begins.
                # Big (512-free) junk matmuls anchor on the x-piece arrivals
                # for ~70% duty; small ones ride the phase-0 op completions.
                def junk(dep=None, n=1, big=False):
                    for _ in range(n):
                        if big:
                            jm = nc.tensor.matmul(
                                av_slot, junk_bf, junk_mv,
                                start=True, stop=True,
                            )
                        else:
                            jm = nc.tensor.matmul(
                                av_slot[:, 0:128], junk_bf, junk_bf,
                                start=True, stop=True,
                            )
                        if dep is not None:
                            add_dep_helper(jm.ins, dep.ins, sync=True,
                                           reason="HAM trickle")

                junk(n=NWARM)
                for dins in xdmas:
                    junk(dep=dins, n=4, big=True)

                # ---------------- phase 0: group stats -> folded affine -----
                # chunk 0 stats on the vector engine (bn_stats),
                # chunk 1 stats on the scalar engine (accum of x and x^2);
                # x is cast to the fp8e4 DoubleRow layout on the way.
                x_f8 = P.tile([128, NCH, N], F8E4, tag="xf8")
                stats = WK.tile([128, 8, 6], FP32, tag="bnstats")
                xsum_p = WK.tile([128, 8], FP32, tag="xsump")
                for p in range(4):
                    ps = slice(p * 1024, (p + 1) * 1024)
                    a1 = nc.scalar.activation(
                        out=x_f8[:, 1, ps], in_=x_sb[1][:, ps], func=AF.Copy,
                        accum_out=xsum_p[:, p:p + 1],
                    )
                    sq_scr = WK.tile([128, 1024], BF16, tag="sqscr")
                    a2 = nc.scalar.activation(
                        out=sq_scr, in_=x_sb[1][:, ps], func=AF.Square,
                        accum_out=xsum_p[:, 4 + p:5 + p],
                    )
                    b1 = nc.vector.bn_stats(
                        out=stats[:, 2 * p, :],
                        in_=x_sb[0][:, p * 1024:p * 1024 + 512],
                    )
                    b2 = nc.vector.bn_stats(
                        out=stats[:, 2 * p + 1, :],
                        in_=x_sb[0][:, p * 1024 + 512:(p + 1) * 1024],
                    )
                    c1 = nc.vector.tensor_copy(
                        out=x_f8[:, 0, ps], in_=x_sb[0][:, ps]
                    )
                    junk(dep=a1)
                    junk(dep=a2)
                    junk(dep=b2)
                    junk(dep=c1)

                # chunk 0: [mean, E[x^2]] from bn aggregation
                mv = WK.tile([128, 2], FP32, tag="bnmv")
                nc.vector.bn_aggr(out=mv, in_=stats)
                t2_0 = WK.tile([128, 2], FP32, tag="chstat0")
                nc.vector.tensor_copy(out=t2_0[:, 0:1], in_=mv[:, 0:1])
                sq = WK.tile([128, 1], FP32, tag="chsq")
                nc.vector.tensor_mul(out=sq, in0=mv[:, 0:1], in1=mv[:, 0:1])
                nc.vector.tensor_add(out=t2_0[:, 1:2], in0=mv[:, 1:2], in1=sq)
                # chunk 1: sums -> [mean, E[x^2]]
                t2_1 = WK.tile([128, 2], FP32, tag="chstat1")
                sab = WK.tile([128, 4], FP32, tag="sab")
                nc.vector.tensor_add(
                    out=sab[:, 0:2], in0=xsum_p[:, 0:2], in1=xsum_p[:, 2:4]
                )
                nc.vector.tensor_add(
                    out=sab[:, 2:4], in0=xsum_p[:, 4:6], in1=xsum_p[:, 6:8]
                )
                nc.vector.tensor_add(
                    out=t2_1[:, 0:1], in0=sab[:, 0:1], in1=sab[:, 1:2]
                )
                nc.vector.tensor_add(
                    out=t2_1[:, 1:2], in0=sab[:, 2:3], in1=sab[:, 3:4]
                )
                jm0 = nc.vector.tensor_scalar_mul(out=t2_1, in0=t2_1, scalar1=1.0 / N)
                junk(dep=jm0, n=2)

                # group stats: [16, 2] per chunk in adjacent psum columns
                t2 = [t2_0, t2_1]
                for j in range(NCH):
                    nc.tensor.matmul(
                        s2[0][0:16, 2 * j:2 * j + 2], ind16_sb, t2[j],
                        start=True, stop=True,
                    )
                g24 = WK.tile([16, 2, 2], FP32, tag="g24")
                nc.vector.tensor_copy(
                    out=g24.rearrange("p a b -> p (a b)"), in_=s2[0][0:16, 0:4]
                )
                mean2 = g24[:, :, 0]  # [16, 2]: group mean per chunk
                ex2 = g24[:, :, 1]  # [16, 2]: group E[x^2] per chunk

                # rstd = rsqrt(var + eps) via bit-trick seed + 2 Newton steps
                # (keeps Sqrt off the scalar engine: exp table stays loaded)
                msq = WK.tile([16, 2], FP32, tag="msq")
                nc.vector.tensor_mul(out=msq, in0=mean2, in1=mean2)
                gvp = WK.tile([16, 2], FP32, tag="gvp")
                nc.vector.tensor_sub(out=gvp, in0=ex2, in1=msq)
                jm1 = nc.vector.tensor_scalar_add(out=gvp, in0=gvp, scalar1=EPS)
                hv = WK.tile([16, 2], FP32, tag="hv")
                nc.vector.tensor_scalar_mul(out=hv, in0=gvp, scalar1=0.5)
                yr = WK.tile([16, 2], FP32, tag="yrsqrt")
                yu = yr.bitcast(U32)
                # magic - (v>>1) computed as (~(v>>1)) - (0xFFFFFFFF - magic):
                # both steps stay in range (DVE integer ops saturate, they
                # don't wrap)
                nc.vector.tensor_scalar(
                    out=yu, in0=gvp.bitcast(U32), scalar1=1,
                    scalar2=0xFFFFFFFF,
                    op0=ALU.logical_shift_right, op1=ALU.bitwise_xor,
                )
                nc.vector.tensor_scalar_sub(
                    out=yu, in0=yu, scalar1=0xFFFFFFFF - RSQRT_MAGIC
                )
                junk(dep=jm1, n=2)
                for it in range(2):
                    aa = WK.tile([16, 2], FP32, tag=f"nra{it}")
                    nc.vector.tensor_mul(out=aa, in0=yr, in1=yr)
                    nc.vector.tensor_mul(out=aa, in0=aa, in1=hv)
                    nc.vector.tensor_scalar(
                        out=aa, in0=aa, scalar1=-1.0, scalar2=1.5,
                        op0=ALU.mult, op1=ALU.add,
                    )
                    jm2 = nc.vector.tensor_mul(out=yr, in0=yr, in1=aa)
                    junk(dep=jm2, n=2)

                fs_sb, fb_sb = [], []
                for j in range(NCH):
                    gm_r = WK.tile([16, 2], FP32, tag=f"gmr{j}")
                    nc.vector.tensor_copy(out=gm_r[:, 0:1], in_=mean2[:, j:j + 1])
                    nc.vector.tensor_copy(out=gm_r[:, 1:2], in_=yr[:, j:j + 1])
                    ps_bc = s2[1][:, 2 * j:2 * j + 2]
                    nc.tensor.matmul(ps_bc, bcast16_sb, gm_r, start=True, stop=True)
                    mbrb = WK.tile([128, 2], FP32, tag=f"mbrb{j}")
                    nc.vector.tensor_copy(out=mbrb, in_=ps_bc)
                    fs = P.tile([128, 1], FP32, tag=f"fs{j}", name=f"fs{j}")
                    nc.vector.tensor_mul(out=fs, in0=gns_sb[j], in1=mbrb[:, 1:2])
                    tmp = WK.tile([128, 1], FP32, tag=f"fbt{j}")
                    nc.vector.tensor_mul(out=tmp, in0=mbrb[:, 0:1], in1=fs)
                    fb = P.tile([128, 1], FP32, tag=f"fb{j}", name=f"fb{j}")
                    nc.vector.tensor_sub(out=fb, in0=gnb_sb[j], in1=tmp)
                    fs_sb.append(fs)
                    fb_sb.append(fb)

                # fp8e4 DoubleRow weights: w'[c_in, c_out] = wT*fs[c_in]*16
                # (x16 keeps typical weight magnitudes out of the e4m3
                # subnormal range; the 1/16 is folded into the psum drains).
                # k first: it gates the score stream for all tiles.
                wq_dr = P.tile([128, NCH, C], F8E4, tag="wqdr")
                wk_dr = P.tile([128, NCH, C], F8E4, tag="wkdr")
                wvo_dr = P.tile([128, NCH, C], F8E4, tag="wvodr")
                for wdst, wsrc in ((wk_dr, wkT_sb), (wq_dr, wqT_sb),
                                   (wvo_dr, wvoT_sb)):
                    for j in range(NCH):
                        jw = nc.vector.tensor_scalar(
                            out=wdst[:, j, :], in0=wsrc[j],
                            scalar1=fs_sb[j], scalar2=16.0,
                            op0=ALU.mult, op1=ALU.mult,
                        )
                        junk(dep=jw)

                # cQ = Wq @ fb + bq (gates the q drains); cV/bo_eff later.
                cq_sb, cv_sb, boe_sb = [], [], []
                rot_t = [0]

                def tiny_mm(wT, rhs2):
                    ps_c = PSD.tile([128, BLK], FP32, tag="d",
                                    name=f"tiny{rot_t[0]}")[:, 300:301]
                    rot_t[0] += 1
                    nc.tensor.matmul(
                        ps_c, wT[0][:, :], rhs2[0], start=True, stop=False,
                    )
                    nc.tensor.matmul(
                        ps_c, wT[1][:, :], rhs2[1], start=False, stop=True,
                    )
                    return ps_c

                for o in range(NCH):
                    ps_c = tiny_mm(
                        [wqT_sb[0][:, o * 128:(o + 1) * 128],
                         wqT_sb[1][:, o * 128:(o + 1) * 128]],
                        fb_sb,
                    )
                    t = P.tile([128, 1], FP32, tag=f"cq{o}", name=f"cq{o}")
                    nc.vector.tensor_add(out=t, in0=ps_c, in1=bq_sb[o])
                    cq_sb.append(t)

                def emit_cv():
                    for o in range(NCH):
                        ps_c = tiny_mm(
                            [wvT_sb[0][:, o * 128:(o + 1) * 128],
                             wvT_sb[1][:, o * 128:(o + 1) * 128]],
                            fb_sb,
                        )
                        t = P.tile([128, 1], FP32, tag=f"cv{o}", name=f"cv{o}")
                        nc.vector.tensor_add(out=t, in0=ps_c, in1=bv_sb[o])
                        cv_sb.append(t)

                def emit_boe():
                    for o in range(NCH):
                        ps_c = tiny_mm(
                            [woT_sb[0][:, o * 128:(o + 1) * 128],
                             woT_sb[1][:, o * 128:(o + 1) * 128]],
                            cv_sb,
                        )
                        t = P.tile([128, 1], FP32, tag=f"boe{o}", name=f"boe{o}")
                        nc.vector.tensor_add(out=t, in0=ps_c, in1=bo_sb[o])
                        boe_sb.append(t)

                # ---------------- phases 1+2: fused projection/attention ----
                q_f8 = P.tile([128, NCH, N], F8E4, tag="qf8")
                k_f8 = P.tile([128, NCH, N], F8E4, tag="kf8")
                vt_f8 = P.tile([128, MT, C], F8E4, tag="vt")
                e_buf = [
                    P.tile([128, MT, BLK], F8E5, tag=f"ebuf{p}", name=f"ebuf{p}")
                    for p in range(2)
                ]
                e_flat = [t.rearrange("p a b -> p (a b)") for t in e_buf]
                vt_flat = vt_f8.rearrange("p a b -> p (a b)")

                rslot = [0]
                cur_reg = [None]

                def next_region():
                    r = s2[rslot[0] % 3]
                    rslot[0] += 1
                    return r

                def proj_pair(which, nb):
                    # both chunks of one token block claim one full region;
                    # the two drains run on two engines in parallel (the
                    # scalar engine only before the exp spine starts).
                    cs = slice(nb * BLK, (nb + 1) * BLK)
                    reg = next_region()
                    wdr = wq_dr if which == "q" else wk_dr
                    for o in range(NCH):
                        ps = reg[:, o * BLK:(o + 1) * BLK]
                        nc.tensor.matmul(
                            ps, wdr[:, :, o * 128:(o + 1) * 128], x_f8[:, :, cs],
                            start=True, stop=True, perf_mode=DR,
                        )
                        if which == "q":
                            if nb == 0 and o == 0:
                                nc.scalar.activation(
                                    out=q_f8[:, o, cs], in_=ps, func=AF.Identity,
                                    bias=cq_sb[o], scale=1.0 / 16.0,
                                )
                            else:
                                nc.vector.tensor_scalar(
                                    out=q_f8[:, o, cs], in0=ps,
                                    scalar1=1.0 / 16.0, scalar2=cq_sb[o],
                                    op0=ALU.mult, op1=ALU.add,
                                )
                        else:
                            if nb == 0 and o == 0:
                                nc.scalar.activation(
                                    out=k_f8[:, o, cs], in_=ps, func=AF.Copy,
                                    scale=1.0 / 16.0,
                                )
                            else:
                                nc.vector.tensor_scalar_mul(
                                    out=k_f8[:, o, cs], in0=ps, scalar1=1.0 / 16.0
                                )

                def emit_score(nb, k):
                    # one DoubleRow matmul per 128-key tile (contraction 256);
                    # exp drains two tiles at once from a [128, 1024] region.
                    ms = slice(k * 128, (k + 1) * 128)
                    cs = slice(nb * BLK, (nb + 1) * BLK)
                    half = k % 2
                    if half == 0:
                        cur_reg[0] = next_region()
                    reg = cur_reg[0]
                    nc.tensor.matmul(
                        reg[:, half * BLK:(half + 1) * BLK],
                        k_f8[:, :, ms], q_f8[:, :, cs],
                        start=True, stop=True, perf_mode=DR,
                    )
                    if half == 1:
                        nc.scalar.activation(
                            out=e_flat[nb % 2][:, (k - 1) * BLK:(k + 1) * BLK],
                            in_=reg, func=AF.Exp, scale=SCALE,
                        )

                def v_pair(i):
                    # two v tiles into the av bank, one [128, 512] drain
                    for h in range(2):
                        k = 2 * i + h
                        ms = slice(k * 128, (k + 1) * 128)
                        nc.tensor.matmul(
                            av_slot[:, h * C:(h + 1) * C], x_f8[:, :, ms],
                            wvo_dr, start=True, stop=True, perf_mode=DR,
                        )
                    nc.vector.tensor_scalar_mul(
                        out=vt_flat[:, 2 * i * C:(2 * i + 2) * C],
                        in0=av_slot, scalar1=1.0 / 16.0,
                    )

                # ---- epoch 0: block-0 scores/exp + all projections ----
                proj_pair("k", 0)
                proj_pair("q", 0)
                for k in range(MT):
                    emit_score(0, k)
                    if k % 2 == 1:
                        v_pair(k // 2)
                    if k % 4 == 3 and k < 28:
                        proj_pair("k", (k + 1) // 4)
                    if k == 25:
                        emit_cv()
                    if k == 27:
                        emit_boe()
                    if k == 29:
                        proj_pair("q", 1)
                    if k == 31:
                        proj_pair("q", 2)

                # ---- epochs 1..8: consume block c = j-1, produce block j ----
                av_sb_cur = [None, None]
                rb_cur = [None]
                xb_cur = [None, None]

                def av_group(j, o, slot):
                    eb = e_buf[(j - 1) % 2]
                    for kp in range(NKP):
                        nc.tensor.matmul(
                            slot,
                            vt_f8[:, 2 * kp:2 * kp + 2, o * 128:(o + 1) * 128],
                            eb[:, 2 * kp:2 * kp + 2, :],
                            start=(kp == 0), stop=(kp == NKP - 1),
                            perf_mode=DR,
                        )

                def av_drain(o, slot):
                    t = WK.tile([128, BLK], BF16, tag=f"avsb{o}", name=f"avsb{o}")
                    nc.vector.tensor_copy(out=t, in_=slot)
                    av_sb_cur[o] = t

                def d_mm(j, i, ps_d):
                    eb = e_buf[(j - 1) % 2]
                    nc.tensor.matmul(
                        ps_d, ones_dr, eb[:, 2 * i:2 * i + 2, :],
                        start=(i == 0), stop=(i == NKP - 1), perf_mode=DR,
                    )

                def d_recip(ps_d):
                    rb = WK.tile([128, BLK], FP32, tag="rbsb")
                    nc.vector.reciprocal_approx_fast(rb, ps_d)
                    rb_cur[0] = rb

                def xb_make(c, o):
                    # xb = (x + bo_eff) / sqrt(2)
                    ccs = slice(c * BLK, (c + 1) * BLK)
                    xb_t = WK.tile([128, BLK], FP32, tag=f"xbt{o}")
                    nc.vector.tensor_scalar(
                        out=xb_t, in0=x_sb[o][:, ccs],
                        scalar1=boe_sb[o], scalar2=INV_SQRT2,
                        op0=ALU.add, op1=ALU.mult,
                    )
                    xb_cur[o] = xb_t

                def y_emit(c, o):
                    # y = x/sqrt2 + bo_eff/sqrt2 + AV'/denom
                    ccs = slice(c * BLK, (c + 1) * BLK)
                    t_t = WK.tile([128, BLK], FP32, tag=f"tt{o}")
                    nc.vector.tensor_tensor(
                        out=t_t, in0=av_sb_cur[o], in1=rb_cur[0], op=ALU.mult
                    )
                    y_t = WK.tile([128, BLK], FP32, tag=f"yt{o}")
                    nc.vector.tensor_add(out=y_t, in0=t_t, in1=xb_cur[o])
                    nc.gpsimd.dma_start(out=y[o * 128:(o + 1) * 128, ccs], in_=y_t)

                def av_mm(j, o, kp, slot):
                    eb = e_buf[(j - 1) % 2]
                    nc.tensor.matmul(
                        slot,
                        vt_f8[:, 2 * kp:2 * kp + 2, o * 128:(o + 1) * 128],
                        eb[:, 2 * kp:2 * kp + 2, :],
                        start=(kp == 0), stop=(kp == NKP - 1),
                        perf_mode=DR,
                    )

                for j in range(1, NBLK + 1):
                    c = j - 1  # consumer block
                    last = j == NBLK
                    ps_d = PSD.tile([128, BLK], FP32, tag="d", name=f"d{j}")
                    if last:
                        # tail: av chunks and the denominator track the last
                        # exps key-pair by key-pair (three open accumulation
                        # groups on three banks; the score ring is dead so
                        # chunk 1 borrows a ring bank), leaving only one
                        # triplet + the y chain after the final exp. The
                        # psum->sbuf casts run on the (now idle) scalar
                        # engine, in parallel with the reciprocal on vector.
                        av1_slot = s2[0][:, 0:BLK]
                        for kp in range(NKP):
                            av_mm(j, 0, kp, av_slot)
                            av_mm(j, 1, kp, av1_slot)
                            d_mm(j, kp, ps_d)
                        d_recip(ps_d)
                        for o, slot in ((0, av_slot), (1, av1_slot)):
                            t = WK.tile([128, BLK], BF16, tag=f"avsb{o}",
                                        name=f"avsb{o}")
                            nc.scalar.activation(out=t, in_=slot, func=AF.Copy)
                            av_sb_cur[o] = t
                            xb_make(c, o)
                            y_emit(c, o)
                        break

                    # interleaved cadence: [s,s,av,av,s,s,av,av,d,d] per
                    # group g -- exp deadlines land evenly and the psum ring
                    # turns over with slack, so producers and consumers zip.
                    for g in range(8):
                        o = 0 if g < 4 else 1
                        kb = 4 * (g % 4)
                        emit_score(j, 4 * g)
                        emit_score(j, 4 * g + 1)
                        av_mm(j, o, kb, av_slot)
                        av_mm(j, o, kb + 1, av_slot)
                        emit_score(j, 4 * g + 2)
                        emit_score(j, 4 * g + 3)
                        av_mm(j, o, kb + 2, av_slot)
                        av_mm(j, o, kb + 3, av_slot)
                        d_mm(j, 2 * g, ps_d)
                        d_mm(j, 2 * g + 1, ps_d)
                        if g == 1 and j <= 5:
                            proj_pair("q", j + 2)
                        if g == 3:
                            av_drain(0, av_slot)
                            xb_make(c, 0)
                        if g == 7:
                            av_drain(1, av_slot)
                            xb_make(c, 1)
                    d_recip(ps_d)
                    y_emit(c, 0)
                    y_emit(c, 1)

    nc.compile()
    return nc


_PROGRAM = None


def _get_program():
    global _PROGRAM
    if _PROGRAM is None:
        _PROGRAM = build_program()
    return _PROGRAM


def make_in_maps(inputs):
    x = np.ascontiguousarray(np.asarray(inputs["x"], dtype=np.float32))
    shared = {
        "wqT": np.ascontiguousarray(np.asarray(inputs["w_q"], np.float32).T),
        "wkT": np.ascontiguousarray(np.asarray(inputs["w_k"], np.float32).T),
        "wvT": np.ascontiguousarray(np.asarray(inputs["w_v"], np.float32).T),
        "woT": np.ascontiguousarray(np.asarray(inputs["w_o"], np.float32).T),
        "wvoT": np.ascontiguousarray(
            (np.asarray(inputs["w_v"], np.float32).T
             @ np.asarray(inputs["w_o"], np.float32).T) * INV_SQRT2
        ),
        "bq": np.asarray(inputs["b_q"], np.float32).reshape(C, 1).copy(),
        "bv": np.asarray(inputs["b_v"], np.float32).reshape(C, 1).copy(),
        "bo": np.asarray(inputs["b_o"], np.float32).reshape(C, 1).copy(),
        "gns": np.asarray(inputs["gn_scale"], np.float32).reshape(C, 1).copy(),
        "gnb": np.asarray(inputs["gn_bias"], np.float32).reshape(C, 1).copy(),
        "ind16": (
            (np.arange(128)[:, None] // GD == np.arange(16)[None, :]) / GD
        ).astype(np.float32),
        "bcast16": (
            np.arange(16)[:, None] == np.arange(128)[None, :] // GD
        ).astype(np.float32),
    }
    in_maps = []
    for i in range(NCORES):
        m = dict(shared)
        m["x"] = np.ascontiguousarray(x[i].reshape(C, N))
        in_maps.append(m)
    return in_maps


def run(inputs, trace=False, trace_cores=None):
    nc = _get_program()
    in_maps = make_in_maps(inputs)
    res = bass_utils.run_bass_kernel_spmd(
        nc, in_maps, core_ids=list(range(NCORES)), trace=trace,
        trace_cores=trace_cores,
    )
    out = np.stack(
        [res.results[i]["y"].reshape(C, H, W) for i in range(NCORES)]
    ).astype(np.float32)
    return out, res


def kernel(**inputs) -> np.ndarray:
    out, _ = run(inputs, trace=False)
    return out
